# revision 1
# baseline (speedup 1.0000x reference)
"""DAGNN Trainium2 kernel: 8-core SPMD, gather-based GCN propagation.

Self-contained: hardcodes shapes for nn_DAGNN_14791867368185.
  x [100000, 512] f32, edge_index [2, 1.6M] i64, K=10,
  W1 [512,256], b1 [256], W2 [256,64], b2 [64], Wp [64,1], bp [1]
Returns (log_softmax(emb), emb) like the reference.

Strategy per hop (per core, nodes sharded 12500/core):
  h' = dinv * h ; AllGather h' (12544-padded blocks -> [100352, 64], pad rows 0)
  4 source buckets of 25088 rows (dma_gather idx is int16).
  Per bucket: destinations sorted by per-bucket in-degree, tiled 128/tile,
  per-tile uniform slot width U (shared schedule across cores, quantized);
  dma_gather slots -> DVE reduce -> partial_b [12544, 64] in bucket order.
  Combine: 4 local gathers (pos_b) + add + dinv scalings -> h, h', pps slot.
MLP encoder and the score/combine/log_softmax epilogue also on device.
"""
import sys
for _p in ('/opt/trn_rl_repo',):
    if _p not in sys.path:
        sys.path.insert(0, _p)
import numpy as np
from concourse import bass, bacc, tile, mybir, bass_utils

NCORES = 8
N = 100000
NLOC = 12500
NLP = 12544            # padded local rows (98 * 128)
NT = 98                # node tiles per core
BSZ = 2 * NLP          # bucket size in padded-global rows = 25088
NB = 4
NG = NCORES * NLP      # 100352
IN_DIM, HID, C, KHOPS = 512, 256, 64, 10
F32, I16 = mybir.dt.float32, mybir.dt.int16
ULEVELS = [1, 2, 3, 4, 5, 6, 8, 10, 12, 16, 20, 24, 32, 48, 64, 96]
GN_MAX = 21            # max tiles per gather call (reduce out width)
GU_MAX = 96            # max slots per partition per call (msg tile width)
G2 = 7                 # node tiles per combine/final group (98 = 14*7)

_prog_cache = {}


def _quant_u(u):
    for lv in ULEVELS:
        if u <= lv:
            return lv
    return ULEVELS[-1]


def _wrap16(vals, pad_cols):
    """int16 idx layout for dma_gather: [128, pad_cols], idx i at
    (i % 16, i // 16), replicated x8 across partition groups."""
    n = len(vals)
    a = np.zeros((16, pad_cols), dtype=np.int16)
    a[np.arange(n) % 16, np.arange(n) // 16] = vals.astype(np.int16)
    return np.tile(a, (8, 1))


def _preprocess(x, edge_index):
    """Host prep: degrees, per-core per-bucket sorted edge slots, schedules."""
    row = np.asarray(edge_index[0], dtype=np.int64)
    col = np.asarray(edge_index[1], dtype=np.int64)
    keep = row != col
    row, col = row[keep], col[keep]
    deg = np.bincount(col, minlength=N).astype(np.float64) + 1.0
    dinv = (1.0 / np.sqrt(deg)).astype(np.float32)

    g_all = (row // NLOC) * NLP + (row % NLOC)   # padded-global src ids

    cores = []
    for c in range(NCORES):
        lo, hi = c * NLOC, (c + 1) * NLOC
        m = (col >= lo) & (col < hi)
        dst = (col[m] - lo).astype(np.int64)
        g = g_all[m]
        per_bucket = []
        for b in range(NB):
            mb = (g // BSZ) == b
            d_b = dst[mb]
            r_b = (g[mb] % BSZ).astype(np.int64)
            order = np.argsort(d_b, kind='stable')
            d_b, r_b = d_b[order], r_b[order]
            # CSR over padded dests
            counts = np.bincount(d_b, minlength=NLP).astype(np.int64)
            # self edge for dests whose global id lives in this bucket
            # g(dest) = c*NLP + d ; bucket = c // 2 (since d < NLOC < NLP)
            starts = np.zeros(NLP + 1, np.int64)
            np.cumsum(counts, out=starts[1:])
            per_bucket.append((counts, starts, r_b))
        cores.append((dst, per_bucket))

    # per-core per-bucket widths including self edge
    Wdb = np.zeros((NCORES, NB, NLP), np.int64)
    for c in range(NCORES):
        _, pb = cores[c]
        for b in range(NB):
            Wdb[c, b] = pb[b][0]
        Wdb[c, c // 2, :NLOC] += 1  # self edge

    # per-bucket dest order (per core) and shared U schedule
    pos = np.zeros((NCORES, NB, NLP), np.int64)     # dest -> slot row in part_b
    invord = np.zeros((NCORES, NB, NLP), np.int64)  # slot row -> dest
    for c in range(NCORES):
        for b in range(NB):
            o = np.argsort(-Wdb[c, b], kind='stable')
            invord[c, b] = o
            pos[c, b, o] = np.arange(NLP)

    Usched = np.zeros((NB, NT), np.int64)
    for b in range(NB):
        for t in range(NT):
            mx = 0
            for c in range(NCORES):
                w = Wdb[c, b][invord[c, b, t * 128:(t + 1) * 128]]
                mx = max(mx, int(w.max()) if len(w) else 0)
            Usched[b, t] = _quant_u(max(mx, 1))

    # gather calls: runs of equal U, capped at CALL_MAX rows
    calls = []  # (bucket, tile_start, n_tiles, U, stream_col_offset)
    col_off = [0] * NB
    for b in range(NB):
        t = 0
        off = 0
        while t < NT:
            U = int(Usched[b, t])
            gcap = min(GN_MAX, GU_MAX // U) if U <= GU_MAX else 1
            g_run = 1
            while (t + g_run < NT and Usched[b, t + g_run] == U
                   and g_run + 1 <= gcap):
                g_run += 1
            calls.append((b, t, g_run, U, off))
            off += g_run * 128 * U // 16   # idx cols consumed
            t += g_run
        col_off[b] = off

    # build idx streams per core
    sidx = [[None] * NB for _ in range(NCORES)]
    cidx = np.zeros((NCORES, NB, NLP), np.int16)
    zbase = 12500  # bucket-relative zero rows: [12500..12543] of first block
    for c in range(NCORES):
        _, pb = cores[c]
        for b in range(NB):
            counts, starts, r_b = pb[b]
            total_rows = sum(128 * int(Usched[b, t]) for t in range(NT))
            stream = np.zeros(total_rows, np.int64)
            si = 0
            self_b = (c // 2 == b)
            for t in range(NT):
                U = int(Usched[b, t])
                dests = invord[c, b, t * 128:(t + 1) * 128]
                blk = np.full((U, 128), -1, np.int64)
                for p in range(128):
                    d = dests[p]
                    w = counts[d]
                    sl = r_b[starts[d]:starts[d] + w]
                    if self_b and d < NLOC:
                        sl = np.concatenate([sl, [(c % 2) * NLP + d]])
                    blk[:len(sl), p] = sl
                fill = blk < 0
                if fill.any():
                    ii = si + (np.nonzero(fill.ravel())[0])
                    blk.ravel()[fill.ravel()] = zbase + (ii % 44)
                stream[si:si + U * 128] = blk.ravel()
                si += U * 128
            sidx[c][b] = stream
            cidx[c, b] = pos[c, b].astype(np.int16)
    return dinv, cores, Usched, calls, col_off, sidx, cidx, invord


def _build_program(Usched, calls, col_off):
    nc = bacc.Bacc("TRN2", target_bir_lowering=False, debug=False,
                   num_devices=NCORES, num_swdge_queues=4)
    AP = {}
    AP['xT'] = nc.dram_tensor("xT", [IN_DIM, NLP], F32, kind="ExternalInput")
    AP['W1'] = nc.dram_tensor("W1", [IN_DIM, HID], F32, kind="ExternalInput")
    AP['W2'] = nc.dram_tensor("W2", [HID, C], F32, kind="ExternalInput")
    AP['b1'] = nc.dram_tensor("b1", [HID, 1], F32, kind="ExternalInput")
    AP['b2r'] = nc.dram_tensor("b2r", [128, C], F32, kind="ExternalInput")
    AP['Wpr'] = nc.dram_tensor("Wpr", [128, (KHOPS + 1) * C], F32, kind="ExternalInput")
    AP['bpr'] = nc.dram_tensor("bpr", [128, 1], F32, kind="ExternalInput")
    AP['dinvbc'] = nc.dram_tensor("dinvbc", [NLP, C], F32, kind="ExternalInput")
    for b in range(NB):
        AP[f'sidx{b}'] = nc.dram_tensor(f"sidx{b}", [128, col_off[b]], I16,
                                        kind="ExternalInput")
    AP['cidx'] = nc.dram_tensor("cidx", [NB * 128, NLP // 16], I16,
                                kind="ExternalInput")
    out_ls = nc.dram_tensor("out_ls", [NLOC, C], F32, kind="ExternalOutput")
    out_emb = nc.dram_tensor("out_emb", [NLOC, C], F32, kind="ExternalOutput")

    KP1 = KHOPS + 1
    with tile.TileContext(nc) as tc:
        with tc.tile_pool(name="persist", bufs=1) as pers, \
             tc.tile_pool(name="sbw", bufs=2) as sbw, \
             tc.tile_pool(name="msgp", bufs=2) as msgp, \
             tc.tile_pool(name="finp", bufs=1) as finp, \
             tc.tile_pool(name="cmb", bufs=2) as cmb, \
             tc.tile_pool(name="psum", bufs=2, space="PSUM") as psp, \
             tc.tile_pool(name="psum2", bufs=2, space="PSUM") as psp2, \
             tc.tile_pool(name="dram", bufs=1, space="DRAM") as dram:

            pps = dram.tile([NLP, KP1 * C], F32)
            hp_b = dram.tile([NLP, C], F32)
            hp_full = dram.tile([NG, C], F32)
            parts = [dram.tile([NLP, C], F32, tag=f"part{b}", name=f"part{b}") for b in range(NB)]

            # resident tiles
            w1 = [pers.tile([128, HID], F32, tag=f"w1_{k}", name=f"w1_{k}") for k in range(4)]
            for k in range(4):
                nc.sync.dma_start(out=w1[k][:], in_=AP['W1'][k * 128:(k + 1) * 128, :])
            w2 = [pers.tile([128, C], F32, tag=f"w2_{k}", name=f"w2_{k}") for k in range(2)]
            for k in range(2):
                nc.sync.dma_start(out=w2[k][:], in_=AP['W2'][k * 128:(k + 1) * 128, :])
            b1t = [pers.tile([128, 1], F32, tag=f"b1_{k}", name=f"b1_{k}") for k in range(2)]
            for k in range(2):
                nc.sync.dma_start(out=b1t[k][:], in_=AP['b1'][k * 128:(k + 1) * 128, :])
            b2r = pers.tile([128, C], F32)
            nc.sync.dma_start(out=b2r[:], in_=AP['b2r'][:])
            dinvbc = pers.tile([128, NT * C], F32)
            nc.sync.dma_start(out=dinvbc[:].rearrange("p (g c) -> p g c", g=NT),
                              in_=AP['dinvbc'][:].rearrange("(g p) c -> p g c", p=128))
            cidx_t = pers.tile([128, NB * (NLP // 16)], I16)
            nc.sync.dma_start(out=cidx_t[:].rearrange("p (b w) -> p b w", b=NB),
                              in_=AP['cidx'][:].rearrange("(b p) w -> p b w", p=128))

            # ---------------- MLP encoder ----------------
            # chunks of R node-columns; hidT = relu(W1^T xT chunk); h = hidT^T W2
            chunks = [(i * 512, 512) for i in range(24)] + [(12288, 256)]
            for (n0, R) in chunks:
                xt = [sbw.tile([128, R], F32, tag=f"xt{k}", name=f"xt{k}", bufs=1) for k in range(4)]
                for k in range(4):
                    nc.sync.dma_start(out=xt[k][:],
                                      in_=AP['xT'][k * 128:(k + 1) * 128, n0:n0 + R])
                hid_sb = []
                for hh in range(2):
                    ps = psp.tile([128, R], F32, tag="hidps", name="hidps")
                    for k in range(4):
                        nc.tensor.matmul(out=ps[:], lhsT=w1[k][:, hh * 128:(hh + 1) * 128],
                                         rhs=xt[k][:], start=(k == 0), stop=(k == 3))
                    hs = sbw.tile([128, R], F32, tag=f"hid{hh}", name=f"hid{hh}", bufs=1)
                    nc.scalar.activation(hs[:], ps[:],
                                         mybir.ActivationFunctionType.Relu,
                                         bias=b1t[hh][:])
                    hid_sb.append(hs)
                for m in range(R // 128):
                    ph = psp2.tile([128, C], F32, tag="hps", name="hps")
                    for hh in range(2):
                        nc.tensor.matmul(out=ph[:],
                                         lhsT=hid_sb[hh][:, m * 128:(m + 1) * 128],
                                         rhs=w2[hh][:], start=(hh == 0), stop=(hh == 1))
                    t_idx = n0 // 128 + m
                    h0 = cmb.tile([128, C], F32, tag="h0", name="h0")
                    nc.vector.tensor_add(h0[:], ph[:], b2r[:])
                    hp0 = cmb.tile([128, C], F32, tag="hp0", name="hp0")
                    nc.vector.tensor_mul(hp0[:], h0[:], dinvbc[:, t_idx * C:(t_idx + 1) * C])
                    nc.sync.dma_start(
                        out=pps[t_idx * 128:(t_idx + 1) * 128, 0:C],
                        in_=h0[:])
                    nc.sync.dma_start(
                        out=hp_b[t_idx * 128:(t_idx + 1) * 128, :], in_=hp0[:])

            # ---------------- K propagation hops ----------------
            for k in range(1, KHOPS + 1):
                nc.gpsimd.collective_compute(
                    "AllGather", mybir.AluOpType.bypass,
                    replica_groups=[list(range(NCORES))],
                    ins=[hp_b[:].opt()], outs=[hp_full[:].opt()])
                for ci, (b, t0, gn, U, coff) in enumerate(calls):
                    rows = gn * 128 * U
                    it = sbw.tile([128, GU_MAX * 8], I16, tag="sidx", name="sidx")
                    iv = it[:, :rows // 16]
                    nc.sync.dma_start(out=iv, in_=AP[f'sidx{b}'][:, coff:coff + rows // 16])
                    msg = msgp.tile([128, GU_MAX, C], F32, tag="msg", name="msg")
                    mv = msg[:, :rows // 128, :]
                    nc.gpsimd.dma_gather(
                        out_ap=mv, in_ap=hp_full[b * BSZ:(b + 1) * BSZ, :],
                        idxs_ap=iv,
                        num_idxs=rows, num_idxs_reg=rows, elem_size=C,
                        single_packet=False, queue_num=ci % 4)
                    red = cmb.tile([128, GN_MAX, C], F32, tag="red", name="red")
                    rv = red[:, :gn, :]
                    nc.vector.tensor_reduce(
                        rv, mv.rearrange("p (g u) c -> p g c u", g=gn),
                        mybir.AxisListType.X, mybir.AluOpType.add)
                    nc.sync.dma_start(
                        out=parts[b][t0 * 128:(t0 + gn) * 128, :].rearrange(
                            "(g p) c -> p g c", p=128),
                        in_=rv)
                # combine groups
                for gi in range(NT // G2):
                    t0 = gi * G2
                    ptile = []
                    for b in range(NB):
                        pt = cmb.tile([128, G2, C], F32, tag=f"cg{b}", name=f"cg{b}", bufs=1)
                        nc.gpsimd.dma_gather(
                            out_ap=pt[:], in_ap=parts[b][:],
                            idxs_ap=cidx_t[:, b * (NLP // 16) + t0 * 8: b * (NLP // 16) + (t0 + G2) * 8],
                            num_idxs=G2 * 128, num_idxs_reg=G2 * 128,
                            elem_size=C, single_packet=False, queue_num=b)
                        ptile.append(pt)
                    ssum = cmb.tile([128, G2, C], F32, tag="ssum", name="ssum")
                    nc.vector.tensor_add(ssum[:], ptile[0][:], ptile[1][:])
                    nc.vector.tensor_add(ssum[:], ssum[:], ptile[2][:])
                    nc.vector.tensor_add(ssum[:], ssum[:], ptile[3][:])
                    hk = cmb.tile([128, G2, C], F32, tag="hk", name="hk")
                    nc.vector.tensor_mul(hk[:], ssum[:], dinvbc[:, t0 * C:(t0 + G2) * C].rearrange('p (g c) -> p g c', g=G2))
                    nc.sync.dma_start(
                        out=pps[t0 * 128:(t0 + G2) * 128, k * C:(k + 1) * C]
                            .rearrange("(g p) c -> p g c", p=128),
                        in_=hk[:])
                    if k < KHOPS:
                        hpk = cmb.tile([128, G2, C], F32, tag="hpk", name="hpk")
                        nc.vector.tensor_mul(hpk[:], hk[:], dinvbc[:, t0 * C:(t0 + G2) * C].rearrange('p (g c) -> p g c', g=G2))
                        nc.sync.dma_start(
                            out=hp_b[t0 * 128:(t0 + G2) * 128, :].rearrange(
                                "(g p) c -> p g c", p=128),
                            in_=hpk[:])

            # ---------------- score / combine / log_softmax ----------------
            Wpr = pers.tile([128, KP1 * C], F32)
            nc.sync.dma_start(out=Wpr[:], in_=AP['Wpr'][:])
            bpr = pers.tile([128, 1], F32)
            nc.sync.dma_start(out=bpr[:], in_=AP['bpr'][:])
            for gi in range(NT // G2):
                t0 = gi * G2
                pb = finp.tile([128, G2, KP1 * C], F32, tag="pb", name="pb")
                nc.sync.dma_start(
                    out=pb[:],
                    in_=pps[t0 * 128:(t0 + G2) * 128, :].rearrange(
                        "(g p) c -> p g c", p=128))
                tmp = finp.tile([128, G2, KP1 * C], F32, tag="ftmp", name="ftmp")
                a1, a2 = bass.broadcast_tensor_aps(
                    pb[:], Wpr[:].rearrange("p (o c) -> p o c", o=1))
                nc.vector.tensor_mul(tmp[:], a1, a2)
                logit = cmb.tile([128, G2 * KP1], F32, tag="logit", name="logit")
                nc.vector.tensor_reduce(
                    logit[:], tmp[:].rearrange("p g (k c) -> p (g k) c", k=KP1),
                    mybir.AxisListType.X, mybir.AluOpType.add)
                score = cmb.tile([128, G2 * KP1], F32, tag="score", name="score")
                nc.scalar.activation(score[:], logit[:],
                                     mybir.ActivationFunctionType.Sigmoid,
                                     bias=bpr[:])
                a1, a2 = bass.broadcast_tensor_aps(
                    pb[:].rearrange("p g (k c) -> p (g k) c", k=KP1),
                    score[:].rearrange("p (a b) -> p a b", b=1))
                nc.vector.tensor_mul(
                    tmp[:].rearrange("p g (k c) -> p (g k) c", k=KP1), a1, a2)
                emb = cmb.tile([128, G2, C], F32, tag="emb", name="emb")
                nc.vector.tensor_reduce(
                    emb[:], tmp[:].rearrange("p g (k c) -> p g c k", k=KP1),
                    mybir.AxisListType.X, mybir.AluOpType.add)
                # log_softmax over C
                rmax = cmb.tile([128, G2], F32, tag="rmax", name="rmax")
                nc.vector.tensor_reduce(rmax[:], emb[:], mybir.AxisListType.X,
                                        mybir.AluOpType.max)
                shift = cmb.tile([128, G2, C], F32, tag="shift", name="shift")
                a1, a2 = bass.broadcast_tensor_aps(
                    emb[:], rmax[:].rearrange("p (g o) -> p g o", o=1))
                nc.vector.tensor_sub(shift[:], a1, a2)
                expd = cmb.tile([128, G2, C], F32, tag="expd", name="expd")
                nc.scalar.activation(expd[:], shift[:],
                                     mybir.ActivationFunctionType.Exp)
                ssum = cmb.tile([128, G2], F32, tag="esum", name="esum")
                nc.vector.tensor_reduce(ssum[:], expd[:], mybir.AxisListType.X,
                                        mybir.AluOpType.add)
                lsum = cmb.tile([128, G2], F32, tag="lsum", name="lsum")
                nc.scalar.activation(lsum[:], ssum[:],
                                     mybir.ActivationFunctionType.Ln)
                lsm = cmb.tile([128, G2, C], F32, tag="lsm", name="lsm")
                a1, a2 = bass.broadcast_tensor_aps(
                    shift[:], lsum[:].rearrange("p (g o) -> p g o", o=1))
                nc.vector.tensor_sub(lsm[:], a1, a2)
                # stores (handle ragged final tile: rows >= NLOC invalid)
                n0, n1 = t0 * 128, (t0 + G2) * 128
                if n1 <= NLOC:
                    nc.sync.dma_start(
                        out=out_emb[n0:n1, :].rearrange("(g p) c -> p g c", p=128),
                        in_=emb[:])
                    nc.sync.dma_start(
                        out=out_ls[n0:n1, :].rearrange("(g p) c -> p g c", p=128),
                        in_=lsm[:])
                else:
                    gfull = (NLOC - n0) // 128          # full tiles in group
                    rem = NLOC - n0 - gfull * 128       # partial rows (84)
                    if gfull:
                        nc.sync.dma_start(
                            out=out_emb[n0:n0 + gfull * 128, :].rearrange(
                                "(g p) c -> p g c", p=128),
                            in_=emb[:, :gfull, :])
                        nc.sync.dma_start(
                            out=out_ls[n0:n0 + gfull * 128, :].rearrange(
                                "(g p) c -> p g c", p=128),
                            in_=lsm[:, :gfull, :])
                    if rem:
                        nc.sync.dma_start(
                            out=out_emb[n0 + gfull * 128:NLOC, :].rearrange(
                                "(o p) c -> p o c", p=rem),
                            in_=emb[:rem, gfull:gfull + 1, :])
                        nc.sync.dma_start(
                            out=out_ls[n0 + gfull * 128:NLOC, :].rearrange(
                                "(o p) c -> p o c", p=rem),
                            in_=lsm[:rem, gfull:gfull + 1, :])
    nc.compile()
    return nc


def kernel(x, edge_index, K, W1, b1, W2, b2, Wp, bp):
    x = np.asarray(x, dtype=np.float32)
    edge_index = np.asarray(edge_index)
    W1 = np.asarray(W1, np.float32); b1 = np.asarray(b1, np.float32)
    W2 = np.asarray(W2, np.float32); b2 = np.asarray(b2, np.float32)
    Wp = np.asarray(Wp, np.float32); bp = np.asarray(bp, np.float32)
    assert int(K) == KHOPS and x.shape == (N, IN_DIM)

    dinv, cores, Usched, calls, col_off, sidx, cidx, invord = \
        _preprocess(x, edge_index)

    key = (tuple(Usched.ravel()), tuple(col_off))
    if key not in _prog_cache:
        _prog_cache[key] = _build_program(Usched, calls, col_off)
    nc = _prog_cache[key]

    Wp_rep = np.tile(Wp[:, 0][None, :], (128, KHOPS + 1)).astype(np.float32)
    bp_rep = np.full((128, 1), float(bp[0]), np.float32)
    b2_rep = np.tile(b2[None, :], (128, 1)).astype(np.float32)

    in_maps = []
    for c in range(NCORES):
        lo = c * NLOC
        xT = np.zeros((IN_DIM, NLP), np.float32)
        xT[:, :NLOC] = x[lo:lo + NLOC].T
        dbc = np.zeros((NLP, C), np.float32)
        dbc[:NLOC] = dinv[lo:lo + NLOC][:, None]
        m = {"xT": xT, "W1": W1, "W2": W2,
             "b1": b1[:, None].astype(np.float32),
             "b2r": b2_rep, "Wpr": Wp_rep, "bpr": bp_rep, "dinvbc": dbc}
        for b in range(NB):
            m[f"sidx{b}"] = _wrap16(sidx[c][b], col_off[b])
        m["cidx"] = np.concatenate(
            [_wrap16(cidx[c][b], NLP // 16) for b in range(NB)], axis=0)
        in_maps.append(m)

    res = bass_utils.run_bass_kernel_spmd(nc, in_maps, list(range(NCORES)))
    ls = np.concatenate([res.results[c]["out_ls"] for c in range(NCORES)], axis=0)
    emb = np.concatenate([res.results[c]["out_emb"] for c in range(NCORES)], axis=0)
    return ls, emb



# revision 9
# speedup vs baseline: 1.8755x; 1.8755x over previous
"""DAGNN Trainium2 kernel: 8-core SPMD, gather-based GCN propagation v2.

Self-contained: hardcodes shapes for nn_DAGNN_14791867368185.
  x [100000, 512] f32, edge_index [2, 1.6M] i64, K=10,
  W1 [512,256], b1 [256], W2 [256,64], b2 [64], Wp [64,1], bp [1]
Returns (log_softmax(emb), emb) like the reference.

v2 design (GpSimd descriptor-generation was the v1 bottleneck):
  - All device state in a per-core canonical order = bucket-0 degree sort.
    Host permutes xT/dinv in, un-permutes outputs after.
  - Self edge handled by DVE (h_next = dinv*(P + h'_prev)), not gathered.
  - Per-bucket degree-sorted gathers with fine U levels (~1.04x edges).
  - Bucket partials land via dma_scatter_add (b=1..3) / plain DMA (b=0)
    into per-bucket comb arrays; DVE sums them. No per-hop combine gathers.
  - Gather/scatter calls LPT-balanced across all 4 SWDGE queues with
    per-queue tile tags so Q7 descriptor generation runs 4-wide.
  - AllGather output in Shared address space.
"""
import sys
for _p in ('/opt/trn_rl_repo',):
    if _p not in sys.path:
        sys.path.insert(0, _p)
import numpy as np
from concourse import bass, bacc, tile, mybir, bass_utils

NCORES = 8
N = 100000
NLOC = 12500
NLP = 12544            # padded local rows (98 * 128)
NT = 98                # node tiles per core
BSZ = 2 * NLP          # bucket size in padded-global rows = 25088
NB = 4
NG = NCORES * NLP      # 100352
IN_DIM, HID, C, KHOPS = 512, 256, 64, 10
F32, I16 = mybir.dt.float32, mybir.dt.int16
ULEVELS = list(range(1, 33)) + [36, 40, 48, 64, 96]
ROWS_MAX = 4096        # max gathered rows per dma_gather call
TILES_MAX = 16         # max dest tiles per call (red tile width)
G2 = 7                 # node tiles per combine group (98 = 14*7)
G2E = 2                # node tiles per epilogue group (98 = 49*2)
NQ = 4                 # SWDGE queues

_prog_cache = {}


def _quant_u(u):
    for lv in ULEVELS:
        if u <= lv:
            return lv
    return ULEVELS[-1]


def _wrap16(vals, pad_cols):
    """int16 idx layout for dma_gather/scatter: [128, pad_cols], idx i at
    (i % 16, i // 16), replicated x8 across partition groups."""
    n = len(vals)
    a = np.zeros((16, pad_cols), dtype=np.int16)
    a[np.arange(n) % 16, np.arange(n) // 16] = vals.astype(np.int16)
    return np.tile(a, (8, 1))


def _preprocess(edge_index):
    """Host prep: degrees, canonical order, per-core per-bucket slot streams,
    scatter streams, shared U schedule + call pack."""
    row = np.asarray(edge_index[0], dtype=np.int64)
    col = np.asarray(edge_index[1], dtype=np.int64)
    keep = row != col
    row, col = row[keep], col[keep]
    deg = np.bincount(col, minlength=N).astype(np.float64) + 1.0
    dinv = (1.0 / np.sqrt(deg)).astype(np.float32)

    # per-core per-bucket degree matrices and sort orders
    Wdb = np.zeros((NCORES, NB, NLP), np.int64)
    order = np.zeros((NCORES, NB, NLP), np.int64)   # slot j -> natural local d
    pos = np.zeros((NCORES, NB, NLP), np.int64)     # natural local d -> slot j
    core_edges = []
    for c in range(NCORES):
        lo, hi = c * NLOC, (c + 1) * NLOC
        m = (col >= lo) & (col < hi)
        core_edges.append((row[m], (col[m] - lo)))
        for b in range(NB):
            pass
    # source canonical rows need pos[cs][0] (bucket-0 order) of source core
    for c in range(NCORES):
        src, dst = core_edges[c]
        for b in range(NB):
            mb = (src // (2 * NLOC)) == b
            Wdb[c, b] = np.bincount(dst[mb], minlength=NLP)
        for b in range(NB):
            o = np.argsort(-Wdb[c, b], kind='stable')
            order[c, b] = o
            pos[c, b, o] = np.arange(NLP)

    ord0 = order[:, 0, :]   # canonical order per core
    pos0 = pos[:, 0, :]

    # map each edge source to its padded-global canonical row
    def srow_of(src):
        cs = src // NLOC
        l = src % NLOC
        return cs * NLP + pos0[cs, l]

    # shared U schedule per (bucket, tile): max over cores
    Usched = np.zeros((NB, NT), np.int64)
    for b in range(NB):
        for t in range(NT):
            mx = 1
            for c in range(NCORES):
                w = Wdb[c, b][order[c, b, t * 128:(t + 1) * 128]]
                mx = max(mx, int(w.max()))
            Usched[b, t] = _quant_u(mx)

    # pack calls: per bucket, consecutive tiles, rows<=ROWS_MAX, tiles<=TILES_MAX
    calls = []  # dict: b, t0, ntiles, us(list), runs[(U,gn)...], rows, coff
    col_off = [0] * NB
    for b in range(NB):
        t = 0
        off = 0
        while t < NT:
            us = []
            rows = 0
            t0 = t
            while t < NT and len(us) < TILES_MAX:
                u = int(Usched[b, t])
                if us and rows + 128 * u > ROWS_MAX:
                    break
                us.append(u)
                rows += 128 * u
                t += 1
            runs = []
            for u in us:
                if runs and runs[-1][0] == u:
                    runs[-1][1] += 1
                else:
                    runs.append([u, 1])
            calls.append(dict(b=b, t0=t0, ntiles=len(us), us=us,
                              runs=[tuple(r) for r in runs], rows=rows,
                              coff=off))
            off += rows // 16
        col_off[b] = off

    # LPT queue assignment by Q7 weight (gather rows + scatter rows)
    wq = [0] * NQ
    for cl in sorted(calls, key=lambda d: -(d['rows'] + (cl_sc(d)))):
        q = min(range(NQ), key=lambda i: wq[i])
        cl['q'] = q
        wq[q] += cl['rows'] + cl_sc(cl)
    # emission order: round-robin across queues
    per_q = [[] for _ in range(NQ)]
    for cl in calls:
        per_q[cl['q']].append(cl)
    emit = []
    step = 0
    while any(per_q):
        for q in range(NQ):
            if per_q[q]:
                emit.append(per_q[q].pop(0))
        step += 1

    # build idx streams + scatter streams per core
    sidx = [[None] * NB for _ in range(NCORES)]
    scat = [[None] * NB for _ in range(NCORES)]
    for c in range(NCORES):
        src, dst = core_edges[c]
        srow = srow_of(src)
        for b in range(NB):
            mb = (srow // BSZ) == b
            d_b = dst[mb]
            r_b = (srow[mb] % BSZ).astype(np.int64)
            o = np.argsort(d_b, kind='stable')
            d_b, r_b = d_b[o], r_b[o]
            counts = np.bincount(d_b, minlength=NLP).astype(np.int64)
            starts = np.zeros(NLP + 1, np.int64)
            np.cumsum(counts, out=starts[1:])
            # always-zero bucket-relative rows (pads of cores 2b, 2b+1)
            zr = []
            for cc in (2 * b, 2 * b + 1):
                zr.extend(((cc % 2) * NLP + pos0[cc, NLOC:NLP]).tolist())
            zr = np.array(zr, np.int64)
            total_rows = sum(128 * int(Usched[b, t]) for t in range(NT))
            stream = np.zeros(total_rows, np.int64)
            si = 0
            for t in range(NT):
                U = int(Usched[b, t])
                dests = order[c, b, t * 128:(t + 1) * 128]
                blk = np.full((U, 128), -1, np.int64)
                for p in range(128):
                    d = dests[p]
                    w = counts[d]
                    sl = r_b[starts[d]:starts[d] + w]
                    blk[:w, p] = sl
                fill = blk < 0
                if fill.any():
                    ii = si + (np.nonzero(fill.ravel())[0])
                    blk.ravel()[fill.ravel()] = zr[ii % len(zr)]
                stream[si:si + U * 128] = blk.ravel()
                si += U * 128
            sidx[c][b] = stream
            if b > 0:
                # scatter: slot j (bucket-b order) -> canonical pos0 of dest
                scat[c][b] = pos0[c, order[c, b]].astype(np.int16)
    return dinv, ord0, Usched, calls, emit, col_off, sidx, scat


def cl_sc(cl):
    return cl['ntiles'] * 128 if cl['b'] > 0 else 0


def _build_program(Usched, emit, col_off):
    nc = bacc.Bacc("TRN2", target_bir_lowering=False, debug=False,
                   num_devices=NCORES, num_swdge_queues=NQ)
    AP = {}
    AP['xT'] = nc.dram_tensor("xT", [IN_DIM, NLP], F32, kind="ExternalInput")
    AP['W1'] = nc.dram_tensor("W1", [IN_DIM, HID], F32, kind="ExternalInput")
    AP['W2'] = nc.dram_tensor("W2", [HID, C], F32, kind="ExternalInput")
    AP['b1'] = nc.dram_tensor("b1", [HID, 1], F32, kind="ExternalInput")
    AP['b2r'] = nc.dram_tensor("b2r", [128, C], F32, kind="ExternalInput")
    AP['Wpr'] = nc.dram_tensor("Wpr", [128, (KHOPS + 1) * C], F32, kind="ExternalInput")
    AP['bpr'] = nc.dram_tensor("bpr", [128, 1], F32, kind="ExternalInput")
    AP['dinvt'] = nc.dram_tensor("dinvt", [128, NT], F32, kind="ExternalInput")
    for b in range(NB):
        AP[f'sidx{b}'] = nc.dram_tensor(f"sidx{b}", [128, col_off[b]], I16,
                                        kind="ExternalInput")
    for b in range(1, NB):
        AP[f'scat{b}'] = nc.dram_tensor(f"scat{b}", [128, NLP // 16], I16,
                                        kind="ExternalInput")
    out_ls = nc.dram_tensor("out_ls", [NLP, C], F32, kind="ExternalOutput")
    out_emb = nc.dram_tensor("out_emb", [NLP, C], F32, kind="ExternalOutput")

    KP1 = KHOPS + 1
    max_cols = max(cl['rows'] // 128 for cl in emit)
    max_idx_cols = max(cl['rows'] // 16 for cl in emit)
    with tile.TileContext(nc) as tc:
        with tc.tile_pool(name="persist", bufs=1) as pers, \
             tc.tile_pool(name="sbw", bufs=2) as sbw, \
             tc.tile_pool(name="msgp", bufs=2) as msgp, \
             tc.tile_pool(name="idxp", bufs=3) as idxp, \
             tc.tile_pool(name="finp", bufs=1) as finp, \
             tc.tile_pool(name="cmb", bufs=2) as cmb, \
             tc.tile_pool(name="psum", bufs=2, space="PSUM") as psp, \
             tc.tile_pool(name="psum2", bufs=2, space="PSUM") as psp2, \
             tc.tile_pool(name="dram", bufs=1, space="DRAM") as dram:

            pps = dram.tile([NLP, KP1 * C], F32)
            hp_b = dram.tile([NLP, C], F32)
            hp_fulls = [dram.tile([NG, C], F32, addr_space="Shared",
                                  tag=f"hpf{k}", name=f"hpf{k}")
                        for k in range(KHOPS)]
            combs = [dram.tile([NLP, C], F32, tag=f"comb{b}", name=f"comb{b}")
                     for b in range(NB)]

            # resident tiles
            w1 = [pers.tile([128, HID], F32, tag=f"w1_{k}", name=f"w1_{k}") for k in range(4)]
            for k in range(4):
                nc.sync.dma_start(out=w1[k][:], in_=AP['W1'][k * 128:(k + 1) * 128, :])
            w2 = [pers.tile([128, C], F32, tag=f"w2_{k}", name=f"w2_{k}") for k in range(2)]
            for k in range(2):
                nc.sync.dma_start(out=w2[k][:], in_=AP['W2'][k * 128:(k + 1) * 128, :])
            b1t = [pers.tile([128, 1], F32, tag=f"b1_{k}", name=f"b1_{k}") for k in range(2)]
            for k in range(2):
                nc.sync.dma_start(out=b1t[k][:], in_=AP['b1'][k * 128:(k + 1) * 128, :])
            b2r = pers.tile([128, C], F32)
            nc.sync.dma_start(out=b2r[:], in_=AP['b2r'][:])
            dinvt = pers.tile([128, NT], F32)
            nc.sync.dma_start(out=dinvt[:], in_=AP['dinvt'][:])
            zero_t = pers.tile([128, 14, C], F32)
            nc.vector.memset(zero_t[:], 0.0)

            # ---------------- MLP encoder ----------------
            chunks = [(i * 512, 512) for i in range(24)] + [(12288, 256)]
            for (n0, R) in chunks:
                xt = [sbw.tile([128, R], F32, tag=f"xt{k}", name=f"xt{k}", bufs=2) for k in range(4)]
                for k in range(4):
                    nc.sync.dma_start(out=xt[k][:],
                                      in_=AP['xT'][k * 128:(k + 1) * 128, n0:n0 + R])
                hid_sb = []
                for hh in range(2):
                    ps = psp.tile([128, R], F32, tag="hidps", name="hidps")
                    for k in range(4):
                        nc.tensor.matmul(out=ps[:], lhsT=w1[k][:, hh * 128:(hh + 1) * 128],
                                         rhs=xt[k][:], start=(k == 0), stop=(k == 3))
                    hs = sbw.tile([128, R], F32, tag=f"hid{hh}", name=f"hid{hh}", bufs=2)
                    nc.scalar.activation(hs[:], ps[:],
                                         mybir.ActivationFunctionType.Relu,
                                         bias=b1t[hh][:])
                    hid_sb.append(hs)
                for m in range(R // 128):
                    ph = psp2.tile([128, C], F32, tag="hps", name="hps")
                    for hh in range(2):
                        nc.tensor.matmul(out=ph[:],
                                         lhsT=hid_sb[hh][:, m * 128:(m + 1) * 128],
                                         rhs=w2[hh][:], start=(hh == 0), stop=(hh == 1))
                    t_idx = n0 // 128 + m
                    h0 = cmb.tile([128, C], F32, tag="h0", name="h0")
                    nc.vector.tensor_add(h0[:], ph[:], b2r[:])
                    hp0 = cmb.tile([128, C], F32, tag="hp0", name="hp0")
                    a1, a2 = bass.broadcast_tensor_aps(
                        h0[:].rearrange("p (g c) -> p g c", g=1),
                        dinvt[:, t_idx:t_idx + 1].rearrange("p (g o) -> p g o", o=1))
                    nc.vector.tensor_mul(hp0[:].rearrange("p (g c) -> p g c", g=1), a1, a2)
                    nc.sync.dma_start(
                        out=pps[t_idx * 128:(t_idx + 1) * 128, 0:C],
                        in_=h0[:])
                    nc.sync.dma_start(
                        out=hp_b[t_idx * 128:(t_idx + 1) * 128, :], in_=hp0[:])

            # ---------------- K propagation hops ----------------
            for k in range(1, KHOPS + 1):
                hp_full = hp_fulls[k - 1]
                nc.gpsimd.collective_compute(
                    "AllGather", mybir.AluOpType.bypass,
                    replica_groups=[list(range(NCORES))],
                    ins=[hp_b[:].opt()], outs=[hp_full[:].opt()])
                # zero the scatter-target comb arrays
                for b in range(1, NB):
                    for i in range(7):
                        nc.sync.dma_start(
                            out=combs[b][i * 14 * 128:(i + 1) * 14 * 128, :]
                                .rearrange("(g p) c -> p g c", p=128),
                            in_=zero_t[:])
                for cl in emit:
                    b, t0, ntiles, rows, coff, q = (cl['b'], cl['t0'],
                                                    cl['ntiles'], cl['rows'],
                                                    cl['coff'], cl['q'])
                    it = idxp.tile([128, max_idx_cols], I16,
                                   tag=f"idx{q}", name=f"idx{q}")
                    iv = it[:, :rows // 16]
                    nc.sync.dma_start(out=iv, in_=AP[f'sidx{b}'][:, coff:coff + rows // 16])
                    msg = msgp.tile([128, max_cols, C], F32,
                                    tag=f"msg{q}", name=f"msg{q}")
                    mv = msg[:, :rows // 128, :]
                    nc.gpsimd.dma_gather(
                        out_ap=mv, in_ap=hp_full[b * BSZ:(b + 1) * BSZ, :],
                        idxs_ap=iv,
                        num_idxs=rows, num_idxs_reg=rows, elem_size=C,
                        single_packet=False, queue_num=q)
                    red = msgp.tile([128, TILES_MAX, C], F32,
                                    tag=f"red{q}", name=f"red{q}")
                    rv = red[:, :ntiles, :]
                    off = 0
                    goff = 0
                    for (U, gn) in cl['runs']:
                        nc.vector.tensor_reduce(
                            red[:, goff:goff + gn, :],
                            msg[:, off:off + gn * U, :].rearrange(
                                "p (g u) c -> p g c u", g=gn),
                            mybir.AxisListType.X, mybir.AluOpType.add)
                        off += gn * U
                        goff += gn
                    if b == 0:
                        nc.sync.dma_start(
                            out=combs[0][t0 * 128:(t0 + ntiles) * 128, :]
                                .rearrange("(g p) c -> p g c", p=128),
                            in_=rv)
                    else:
                        st = idxp.tile([128, TILES_MAX * 8], I16,
                                       tag=f"sc{q}", name=f"sc{q}", bufs=2)
                        sv = st[:, :ntiles * 8]
                        nc.sync.dma_start(out=sv,
                                          in_=AP[f'scat{b}'][:, t0 * 8:(t0 + ntiles) * 8])
                        nc.gpsimd.dma_scatter_add(
                            out_ap=combs[b][:], in_ap=rv, idxs_ap=sv,
                            num_idxs=ntiles * 128, num_idxs_reg=ntiles * 128,
                            elem_size=C, single_packet=False, queue_num=q)
                # combine groups
                for gi in range(NT // G2):
                    t0 = gi * G2
                    ct = []
                    for b in range(NB):
                        pt = cmb.tile([128, G2, C], F32, tag=f"cg{b}", name=f"cg{b}")
                        nc.sync.dma_start(
                            out=pt[:],
                            in_=combs[b][t0 * 128:(t0 + G2) * 128, :]
                                .rearrange("(g p) c -> p g c", p=128))
                        ct.append(pt)
                    hprev = cmb.tile([128, G2, C], F32, tag="hprev", name="hprev")
                    nc.sync.dma_start(
                        out=hprev[:],
                        in_=hp_b[t0 * 128:(t0 + G2) * 128, :]
                            .rearrange("(g p) c -> p g c", p=128))
                    s01 = cmb.tile([128, G2, C], F32, tag="s01", name="s01")
                    nc.vector.tensor_add(s01[:], ct[0][:], ct[1][:])
                    s23 = cmb.tile([128, G2, C], F32, tag="s23", name="s23")
                    nc.vector.tensor_add(s23[:], ct[2][:], ct[3][:])
                    nc.vector.tensor_add(s01[:], s01[:], s23[:])
                    nc.vector.tensor_add(s01[:], s01[:], hprev[:])
                    hk = cmb.tile([128, G2, C], F32, tag="hk", name="hk")
                    a1, a2 = bass.broadcast_tensor_aps(
                        s01[:], dinvt[:, t0:t0 + G2].rearrange("p (g o) -> p g o", o=1))
                    nc.vector.tensor_mul(hk[:], a1, a2)
                    nc.sync.dma_start(
                        out=pps[t0 * 128:(t0 + G2) * 128, k * C:(k + 1) * C]
                            .rearrange("(g p) c -> p g c", p=128),
                        in_=hk[:])
                    if k < KHOPS:
                        hpk = cmb.tile([128, G2, C], F32, tag="hpk", name="hpk")
                        a1, a2 = bass.broadcast_tensor_aps(
                            hk[:], dinvt[:, t0:t0 + G2].rearrange("p (g o) -> p g o", o=1))
                        nc.vector.tensor_mul(hpk[:], a1, a2)
                        nc.sync.dma_start(
                            out=hp_b[t0 * 128:(t0 + G2) * 128, :].rearrange(
                                "(g p) c -> p g c", p=128),
                            in_=hpk[:])

            # ---------------- score / combine / log_softmax ----------------
            Wpr = pers.tile([128, KP1 * C], F32)
            nc.sync.dma_start(out=Wpr[:], in_=AP['Wpr'][:])
            bpr = pers.tile([128, 1], F32)
            nc.sync.dma_start(out=bpr[:], in_=AP['bpr'][:])
            for gi in range(NT // G2E):
                t0 = gi * G2E
                pb = finp.tile([128, G2E, KP1 * C], F32, tag="pb", name="pb")
                nc.sync.dma_start(
                    out=pb[:],
                    in_=pps[t0 * 128:(t0 + G2E) * 128, :].rearrange(
                        "(g p) c -> p g c", p=128))
                tmp = finp.tile([128, G2E, KP1 * C], F32, tag="ftmp", name="ftmp")
                a1, a2 = bass.broadcast_tensor_aps(
                    pb[:], Wpr[:].rearrange("p (o c) -> p o c", o=1))
                nc.vector.tensor_mul(tmp[:], a1, a2)
                logit = cmb.tile([128, G2E * KP1], F32, tag="logit", name="logit")
                nc.vector.tensor_reduce(
                    logit[:], tmp[:].rearrange("p g (k c) -> p (g k) c", k=KP1),
                    mybir.AxisListType.X, mybir.AluOpType.add)
                score = cmb.tile([128, G2E * KP1], F32, tag="score", name="score")
                nc.scalar.activation(score[:], logit[:],
                                     mybir.ActivationFunctionType.Sigmoid,
                                     bias=bpr[:])
                a1, a2 = bass.broadcast_tensor_aps(
                    pb[:].rearrange("p g (k c) -> p (g k) c", k=KP1),
                    score[:].rearrange("p (a b) -> p a b", b=1))
                nc.vector.tensor_mul(
                    tmp[:].rearrange("p g (k c) -> p (g k) c", k=KP1), a1, a2)
                emb = cmb.tile([128, G2E, C], F32, tag="emb", name="emb")
                nc.vector.tensor_reduce(
                    emb[:], tmp[:].rearrange("p g (k c) -> p g c k", k=KP1),
                    mybir.AxisListType.X, mybir.AluOpType.add)
                # log_softmax over C
                rmax = cmb.tile([128, G2E], F32, tag="rmax", name="rmax")
                nc.vector.tensor_reduce(rmax[:], emb[:], mybir.AxisListType.X,
                                        mybir.AluOpType.max)
                shift = cmb.tile([128, G2E, C], F32, tag="shift", name="shift")
                a1, a2 = bass.broadcast_tensor_aps(
                    emb[:], rmax[:].rearrange("p (g o) -> p g o", o=1))
                nc.vector.tensor_sub(shift[:], a1, a2)
                expd = cmb.tile([128, G2E, C], F32, tag="expd", name="expd")
                nc.scalar.activation(expd[:], shift[:],
                                     mybir.ActivationFunctionType.Exp)
                ssum = cmb.tile([128, G2E], F32, tag="esum", name="esum")
                nc.vector.tensor_reduce(ssum[:], expd[:], mybir.AxisListType.X,
                                        mybir.AluOpType.add)
                lsum = cmb.tile([128, G2E], F32, tag="lsum", name="lsum")
                nc.scalar.activation(lsum[:], ssum[:],
                                     mybir.ActivationFunctionType.Ln)
                lsm = cmb.tile([128, G2E, C], F32, tag="lsm", name="lsm")
                a1, a2 = bass.broadcast_tensor_aps(
                    shift[:], lsum[:].rearrange("p (g o) -> p g o", o=1))
                nc.vector.tensor_sub(lsm[:], a1, a2)
                n0, n1 = t0 * 128, (t0 + G2E) * 128
                nc.sync.dma_start(
                    out=out_emb[n0:n1, :].rearrange("(g p) c -> p g c", p=128),
                    in_=emb[:])
                nc.sync.dma_start(
                    out=out_ls[n0:n1, :].rearrange("(g p) c -> p g c", p=128),
                    in_=lsm[:])
    nc.compile()
    return nc


def kernel(x, edge_index, K, W1, b1, W2, b2, Wp, bp):
    x = np.asarray(x, dtype=np.float32)
    edge_index = np.asarray(edge_index)
    W1 = np.asarray(W1, np.float32); b1 = np.asarray(b1, np.float32)
    W2 = np.asarray(W2, np.float32); b2 = np.asarray(b2, np.float32)
    Wp = np.asarray(Wp, np.float32); bp = np.asarray(bp, np.float32)
    assert int(K) == KHOPS and x.shape == (N, IN_DIM)

    dinv, ord0, Usched, calls, emit, col_off, sidx, scat = \
        _preprocess(edge_index)

    key = (tuple(Usched.ravel()), tuple(col_off),
           tuple((cl['b'], cl['t0'], cl['ntiles'], cl['q']) for cl in emit))
    if key not in _prog_cache:
        _prog_cache[key] = _build_program(Usched, emit, col_off)
    nc = _prog_cache[key]

    Wp_rep = np.tile(Wp[:, 0][None, :], (128, KHOPS + 1)).astype(np.float32)
    bp_rep = np.full((128, 1), float(bp[0]), np.float32)
    b2_rep = np.tile(b2[None, :], (128, 1)).astype(np.float32)

    in_maps = []
    for c in range(NCORES):
        lo = c * NLOC
        o = ord0[c]
        valid = o < NLOC
        xT = np.zeros((IN_DIM, NLP), np.float32)
        xT[:, valid] = x[lo + o[valid]].T
        dvec = np.zeros(NLP, np.float32)
        dvec[valid] = dinv[lo + o[valid]]
        dinvt = dvec.reshape(NT, 128).T.copy()   # [128, NT]
        m = {"xT": xT, "W1": W1, "W2": W2,
             "b1": b1[:, None].astype(np.float32),
             "b2r": b2_rep, "Wpr": Wp_rep, "bpr": bp_rep, "dinvt": dinvt}
        for b in range(NB):
            m[f"sidx{b}"] = _wrap16(sidx[c][b], col_off[b])
        for b in range(1, NB):
            m[f"scat{b}"] = _wrap16(scat[c][b], NLP // 16)
        in_maps.append(m)

    res = bass_utils.run_bass_kernel_spmd(nc, in_maps, list(range(NCORES)))
    ls = np.empty((N, C), np.float32)
    emb = np.empty((N, C), np.float32)
    for c in range(NCORES):
        lo = c * NLOC
        o = ord0[c]
        valid = o < NLOC
        ls[lo + o[valid]] = res.results[c]["out_ls"][valid]
        emb[lo + o[valid]] = res.results[c]["out_emb"][valid]
    return ls, emb
